# revision 11
# baseline (speedup 1.0000x reference)
"""Causal self-attention (B=4, T=2048, C=1024, H=16) on 8 TRN2 NeuronCores.

Sharding: tensor-parallel pairs. Core c handles batch b = c//2 and head-half
j = c%2 (8 of the 16 heads). Each core computes the QKV projection for its
heads, causal attention, and the out-projection contracted over its half of
the features, producing a partial output. The pair-sum (the "all-reduce after
out_proj" of the tensor-parallel scheme) happens at unshard time on the host.

Single fused pipeline, all matmuls bf16 (fp32 PSUM accumulate). The QKV
projection and out-projection matmuls are emitted as *filler* between
attention kt-steps so the PE stream stays dense (HAM stays at 8/8) while
ScalarE's exp stream (~155us, the attention-phase bound) overlaps fully.

Causal handling: exact 128-column trim per diagonal tile (lo = d*128), the
1/sqrt(D) scale rides the ACT instruction's free scale field, and the
diagonal triangle of the probs is zeroed on GpSimd after exp (scaled scores
are ~N(0,1) so exp of unmasked garbage can't overflow) -- keeping VectorE
off the scores->exp critical path. Softmax denominators come free from a
ones-column appended to V so the AV matmul accumulates sum(exp) in PSUM.
The per-pair normalization chain (copy denom -> reciprocal -> broadcast ->
scale) is emitted lazily inside the NEXT pair's kt loop so its ~3us latency
hides under scores/filler work instead of stalling the PSUM accumulators.
"""
import ml_dtypes
import numpy as np
from collections import deque
from contextlib import ExitStack

import concourse.bass as bass
from concourse import bacc
import concourse.mybir as mybir
import concourse.tile as tile
from concourse.bass_utils import run_bass_kernel_spmd

B, T, C, H, D = 4, 2048, 1024, 16, 64
NCORES = 8
HPC = H // 2          # heads per core
F = HPC * D           # 512 features per core (per q/k/v)
KI = C // 128         # 8 contraction tiles over C
NT = T // 512         # 4 token chunks
F32 = mybir.dt.float32
BF16 = mybir.dt.bfloat16

_NC_CACHE = None


def _build():
    nc = bacc.Bacc("TRN2", target_bir_lowering=False, debug=False)
    xt = nc.dram_tensor("xt", [C, T], BF16, kind="ExternalInput").ap()
    wqkvt = nc.dram_tensor("wqkvt", [C, 3 * F], BF16, kind="ExternalInput").ap()
    woutt = nc.dram_tensor("woutt", [F, C], BF16, kind="ExternalInput").ap()
    out = nc.dram_tensor("out", [C, T], F32, kind="ExternalOutput").ap()

    with ExitStack() as ctx:
        tc = ctx.enter_context(tile.TileContext(nc))
        qk = ctx.enter_context(tc.tile_pool(name="qk", bufs=1))
        vp = ctx.enter_context(tc.tile_pool(name="vp", bufs=1))
        wqp = ctx.enter_context(tc.tile_pool(name="wqp", bufs=1))
        wop = ctx.enter_context(tc.tile_pool(name="wop", bufs=1))
        xcp = ctx.enter_context(tc.tile_pool(name="xcp", bufs=2))
        pbp = ctx.enter_context(tc.tile_pool(name="pbp", bufs=8))
        rp = ctx.enter_context(tc.tile_pool(name="rp", bufs=4))
        rbp = ctx.enter_context(tc.tile_pool(name="rbp", bufs=4))
        ytp = ctx.enter_context(tc.tile_pool(name="ytp", bufs=2))
        oop = ctx.enter_context(tc.tile_pool(name="oop", bufs=3))
        # PSUM: scores 2x2 banks + AV pair accumulators 2 + shared proj/outproj 2
        sps = ctx.enter_context(tc.tile_pool(name="sps", bufs=2, space="PSUM"))
        yps = ctx.enter_context(tc.tile_pool(name="yps", bufs=1, space="PSUM"))
        gps = ctx.enter_context(tc.tile_pool(name="gps", bufs=2, space="PSUM"))

        # SBUF-resident tensors for the whole kernel:
        #   qT[m][128f, T], kT[m][128f, T] bf16 feature-major
        #   vt[tm][128tk, 583] bf16 token-major, 8 head-groups of 65 cols
        #   (64 v features + ones col), tail-padded so every 128-col FWL
        #   weight window stays in bounds; pad/ones cols only feed psum
        #   partitions >= 65 which are never read.
        qts = [qk.tile([128, T], BF16, tag=f"q{m}", name=f"q{m}") for m in range(4)]
        kts = [qk.tile([128, T], BF16, tag=f"k{m}", name=f"k{m}") for m in range(4)]
        vts = [vp.tile([128, 583], BF16, tag=f"v{tm}", name=f"v{tm}")
               for tm in range(T // 128)]
        wq = [wqp.tile([128, 3 * F], BF16, tag=f"w{ki}", name=f"w{ki}")
              for ki in range(KI)]
        wo = [wop.tile([128, C], BF16, tag=f"wo{ki}", name=f"wo{ki}")
              for ki in range(F // 128)]

        # ---- DMA prologue: q/k weight halves + chunk-0 x first (in use
        # order so the first matmuls start after the first pair lands),
        # v weight halves next; wo is deferred to the qc loop ----
        xc0 = []
        for ki in range(KI):
            nc.sync.dma_start(out=wq[ki][:, 0:2 * F],
                              in_=wqkvt[ki * 128:(ki + 1) * 128, 0:2 * F])
            t = xcp.tile([128, 512], BF16, tag=f"xc{ki}", name=f"xc{ki}")
            nc.sync.dma_start(out=t[:], in_=xt[ki * 128:(ki + 1) * 128, 0:512])
            xc0.append(t)
        for ki in range(KI):
            nc.sync.dma_start(out=wq[ki][:, 2 * F:3 * F],
                              in_=wqkvt[ki * 128:(ki + 1) * 128, 2 * F:3 * F])
        for tm in range(T // 128):
            nc.gpsimd.memset(vts[tm][:], 1.0)

        # ---- projection emitters ----
        def proj_qk(xc, n, ms):
            # q/k feature-major: psum tile per m, ki-major accumulation
            pts = [gps.tile([128, 512], F32, tag="gp", name=f"gp{m}") for m in ms]
            for ki in range(KI):
                for i, m in enumerate(ms):
                    nc.tensor.matmul(pts[i][:], wq[ki][:, m * 128:(m + 1) * 128],
                                     xc[ki][:], start=(ki == 0), stop=(ki == KI - 1))
            for i, m in enumerate(ms):
                dst = (qts[m] if m < 4 else kts[m - 4])[:, n * 512:(n + 1) * 512]
                nc.vector.tensor_copy(dst, pts[i][:])

        def proj_v(xc, n, tmis):
            # v token-major
            pts = [gps.tile([128, 512], F32, tag="gp", name=f"gpv{t}") for t in tmis]
            for ki in range(KI):
                for i, tmi in enumerate(tmis):
                    nc.tensor.matmul(pts[i][:], xc[ki][:, tmi * 128:(tmi + 1) * 128],
                                     wq[ki][:, 2 * F:3 * F],
                                     start=(ki == 0), stop=(ki == KI - 1))
            for i, tmi in enumerate(tmis):
                tm = n * 4 + tmi
                vdst = vts[tm][:, 0:520].rearrange("p (h c) -> p h c", c=65)
                nc.vector.tensor_copy(
                    vdst[:, :, 0:64], pts[i][:].rearrange("p (h c) -> p h c", c=64))

        # ---- filler machinery: PE work to interleave into attention ----
        fillers = deque()          # (cost_ns, fn, tag)

        def pump(ns):
            while ns > 0 and fillers:
                c, f, _ = fillers.popleft()
                f()
                ns -= c

        def flush_tag(tag):
            while any(e[2] == tag for e in fillers):
                _, f, _ = fillers.popleft()
                f()

        def flush():
            while fillers:
                fillers.popleft()[1]()

        def mk_proj_qk(xc, n, m):
            return (KI * 223 + 50, lambda: proj_qk(xc, n, [m]), f"c{n}h{m % 4}")

        def mk_proj_v(xc, n, tmi):
            return (KI * 223 + 50, lambda: proj_v(xc, n, [tmi]), f"c{n}v")

        def mk_outproj(yy, qc, m):
            def emit():
                po = gps.tile([128, 512], F32, tag="gp", name="gpo")
                for ki in range(F // 128):
                    nc.tensor.matmul(po[:], wo[ki][:, m * 128:(m + 1) * 128],
                                     yy[ki][:], start=(ki == 0),
                                     stop=(ki == F // 128 - 1))
                oo = oop.tile([128, 512], F32, tag="oo", name="oo")
                nc.vector.tensor_copy(oo[:], po[:])
                nc.sync.dma_start(
                    out=out[m * 128:(m + 1) * 128, qc * 512:(qc + 1) * 512],
                    in_=oo[:])
            return (4 * 223 + 50, emit, f"op{qc}")

        # chunk-0 projection: hp0's q/k and all of v inline (ki-major 2-psum
        # groups so compute starts with the first DMA pair); remaining q/k
        # groups become fillers gated per head-pair
        proj_qk(xc0, 0, [0, 4])
        proj_v(xc0, 0, [0, 1])
        proj_v(xc0, 0, [2, 3])
        held = {}
        held[0] = [(2 * KI * 223 + 50,
                    (lambda h: lambda: proj_qk(xc0, 0, [h, h + 4]))(hp),
                    f"c0h{hp}") for hp in (1, 2, 3)]

        pending_norm = [None]

        def emit_pending_norm():
            if pending_norm[0] is not None:
                pending_norm[0]()
                pending_norm[0] = None

        # ---- fused attention + interleaved proj/out-proj ----
        for qc in range(NT):
            # deferred q/k projection units of chunk qc (emitted inside this
            # slot, gated per head-pair) -- keeps PE filler available in the
            # ACT-bound late slots
            fillers.extend(held.pop(qc, []))
            if qc + 1 < NT:
                xcn = []
                for ki in range(KI):
                    t = xcp.tile([128, 512], BF16, tag=f"xc{ki}", name=f"xc{ki}")
                    nc.sync.dma_start(
                        out=t[:],
                        in_=xt[ki * 128:(ki + 1) * 128,
                               (qc + 1) * 512:(qc + 2) * 512])
                    xcn.append(t)
                # v + head-pair-0 groups are due by the qc boundary; the
                # other head-pairs' groups are held for slot qc+1 itself
                for tmi in range(4):
                    fillers.append(mk_proj_v(xcn, qc + 1, tmi))
                fillers.append(mk_proj_qk(xcn, qc + 1, 0))
                fillers.append(mk_proj_qk(xcn, qc + 1, 4))
                held[qc + 1] = [mk_proj_qk(xcn, qc + 1, m)
                                for m in (1, 5, 2, 6, 3, 7)]
            if qc == 0:
                # wo needed first by outproj(0), pumped during qc1
                for ki in range(F // 128):
                    nc.sync.dma_start(out=wo[ki][:],
                                      in_=woutt[ki * 128:(ki + 1) * 128, :])

            n_kt = qc * 4 + 4
            yy = [ytp.tile([128, 512], BF16, tag=f"y{i}", name=f"y{i}")
                  for i in range(4)]
            for hp in range(HPC // 2):       # head pairs (2*hp, 2*hp+1)
                if hp > 0:
                    flush_tag(f"c{qc}h{hp}")
                qpair = qts[hp][:, qc * 512:(qc + 1) * 512]
                pyA = yps.tile([128, 512], F32, tag="pyA", name="pyA")
                pyB = yps.tile([128, 512], F32, tag="pyB", name="pyB")

                def emit_av(item, hp=hp, n_kt=n_kt, pyA=pyA, pyB=pyB):
                    kt, lo, pb = item
                    a0 = 2 * hp * 65
                    nc.tensor.matmul(pyA[:, lo:512], vts[kt][:, a0:a0 + 128],
                                     pb[:, 0, lo:512],
                                     start=(kt == 0), stop=(kt == n_kt - 1))
                    nc.tensor.matmul(pyB[:, lo:512], vts[kt][:, a0 + 65:a0 + 193],
                                     pb[:, 1, lo:512],
                                     start=(kt == 0), stop=(kt == n_kt - 1))

                pending_av = deque()
                for kt in range(n_kt):
                    ksl = kts[hp][:, kt * 128:(kt + 1) * 128]
                    # exact causal trim: cols < lo are fully masked
                    lo = max((kt - qc * 4) * 128, 0)
                    ps = sps.tile([128, 2, 512], F32, tag="ps", name="ps")
                    nc.tensor.matmul(ps[:, 0, lo:512], ksl[0:64, :],
                                     qpair[0:64, lo:512],
                                     start=True, stop=True, tile_position=(0, 0))
                    nc.tensor.matmul(ps[:, 1, lo:512], ksl[64:128, :],
                                     qpair[64:128, lo:512],
                                     start=True, stop=True, tile_position=(64, 0))
                    pb = pbp.tile([128, 2, 512], BF16, tag="pb", name="pb")
                    nc.scalar.activation(pb[:, :, lo:512], ps[:, :, lo:512],
                                         mybir.ActivationFunctionType.Exp,
                                         scale=0.125)
                    diag = kt >= qc * 4
                    if diag:
                        # zero the strictly-lower triangle (query < key) of the
                        # diagonal 128x128 block, post-exp, off the DVE path
                        reg = pb[:, :, lo:lo + 128]
                        nc.gpsimd.affine_select(
                            out=reg, in_=reg,
                            compare_op=mybir.AluOpType.is_ge, fill=0.0,
                            base=0, pattern=[[0, 2], [1, 128]],
                            channel_multiplier=-1)
                    pending_av.append((kt, lo, pb))
                    if kt == 1:
                        emit_pending_norm()
                    if len(pending_av) > 2:
                        emit_av(pending_av.popleft())
                    act_ns = (2 * (512 - lo) + 352) / 1.2 + (150 if diag else 0)
                    pe_ns = 3 * (512 - lo) / 2.4 + 60
                    pump(act_ns - pe_ns)
                emit_pending_norm()
                while pending_av:
                    emit_av(pending_av.popleft())

                def norm(hp=hp, qc=qc, pyA=pyA, pyB=pyB, yy=yy):
                    # denominators sit in psum partition 64 (ones-column of V);
                    # custom-DVE recip can't read PSUM on HW: bounce via SBUF
                    rA = rp.tile([1, 512], F32, tag="rA", name="rA")
                    rB = rp.tile([1, 512], F32, tag="rB", name="rB")
                    nc.vector.tensor_copy(rA[:], pyA[64:65, :])
                    nc.vector.tensor_copy(rB[:], pyB[64:65, :])
                    nc.vector.reciprocal_approx_fast(out=rA[:], in_=rA[:])
                    nc.vector.reciprocal_approx_fast(out=rB[:], in_=rB[:])
                    rbA = rbp.tile([64, 512], F32, tag="rbA", name="rbA")
                    rbB = rbp.tile([64, 512], F32, tag="rbB", name="rbB")
                    nc.gpsimd.partition_broadcast(rbA[:], rA[:])
                    nc.gpsimd.partition_broadcast(rbB[:], rB[:])
                    nc.vector.tensor_mul(yy[hp][0:64, :], pyA[0:64, :], rbA[:])
                    nc.vector.tensor_mul(yy[hp][64:128, :], pyB[0:64, :], rbB[:])

                pending_norm[0] = norm
                if hp + 1 < HPC // 2:
                    # next pair's deferred proj, with lead time for its evac
                    flush_tag(f"c{qc}h{hp + 1}")
                pump(1200)
            # chunk qc+1's v and head-pair-0 groups must be complete before
            # slot qc+1 starts; outproj(qc-1) must drain before yy bufs
            # recycle; the last pair's norm must be emitted before outproj(qc)
            # fillers (Tile deps are emission-order based)
            emit_pending_norm()
            if qc + 1 < NT:
                flush_tag(f"c{qc + 1}v")
                flush_tag(f"c{qc + 1}h0")
            if qc > 0:
                flush_tag(f"op{qc - 1}")
            for m in range(8):
                fillers.append(mk_outproj(yy, qc, m))
        flush()
    nc.finalize()
    return nc


def _get_nc():
    global _NC_CACHE
    if _NC_CACHE is None:
        _NC_CACHE = _build()
    return _NC_CACHE


def kernel(x, w_qkv, w_out):
    x = np.ascontiguousarray(np.asarray(x), dtype=np.float32)
    w_qkv = np.asarray(w_qkv, dtype=np.float32)
    w_out = np.asarray(w_out, dtype=np.float32)
    nc = _get_nc()

    in_maps = []
    for c in range(NCORES):
        b, j = divmod(c, 2)
        rows = np.r_[j * F:(j + 1) * F,
                     C + j * F:C + (j + 1) * F,
                     2 * C + j * F:2 * C + (j + 1) * F]
        in_maps.append({
            "xt": np.ascontiguousarray(x[b].T).astype(ml_dtypes.bfloat16),
            "wqkvt": np.ascontiguousarray(w_qkv[rows, :].T).astype(ml_dtypes.bfloat16),
            "woutt": np.ascontiguousarray(
                w_out[:, j * F:(j + 1) * F].T).astype(ml_dtypes.bfloat16),
        })

    res = run_bass_kernel_spmd(nc, in_maps, core_ids=list(range(NCORES)))
    y = np.empty((B, T, C), np.float32)
    for b in range(B):
        y[b] = (res.results[2 * b]["out"] + res.results[2 * b + 1]["out"]).T
    return y


# revision 15
# speedup vs baseline: 1.0277x; 1.0277x over previous
"""Causal self-attention (B=4, T=2048, C=1024, H=16) on 8 TRN2 NeuronCores.

Sharding: tensor-parallel pairs. Core c handles batch b = c//2 and head-half
j = c%2 (8 of the 16 heads). Each core computes the QKV projection for its
heads, causal attention, and the out-projection contracted over its half of
the features, producing a partial output. The pair-sum (the "all-reduce after
out_proj" of the tensor-parallel scheme) happens at unshard time on the host.

Single fused pipeline, all matmuls bf16 (fp32 PSUM accumulate). The QKV
projection and out-projection matmuls are emitted as *filler* between
attention kt-steps so the PE stream stays dense (HAM stays at 8/8) while
ScalarE's exp stream (~155us, the attention-phase bound) overlaps fully.

Causal handling: exact 128-column trim per diagonal tile (lo = d*128), the
1/sqrt(D) scale rides the ACT instruction's free scale field, and the
diagonal triangle of the probs is zeroed on GpSimd after exp (scaled scores
are ~N(0,1) so exp of unmasked garbage can't overflow) -- keeping VectorE
off the scores->exp critical path. Softmax denominators come free from a
ones-column appended to V so the AV matmul accumulates sum(exp) in PSUM.
The per-pair normalization chain (copy denom -> reciprocal -> broadcast ->
scale) is emitted lazily inside the NEXT pair's kt loop so its ~3us latency
hides under scores/filler work instead of stalling the PSUM accumulators.
"""
import ml_dtypes
import numpy as np
from collections import deque
from contextlib import ExitStack

import concourse.bass as bass
from concourse import bacc
import concourse.mybir as mybir
import concourse.tile as tile
from concourse.bass_utils import run_bass_kernel_spmd

B, T, C, H, D = 4, 2048, 1024, 16, 64
NCORES = 8
HPC = H // 2          # heads per core
F = HPC * D           # 512 features per core (per q/k/v)
KI = C // 128         # 8 contraction tiles over C
NT = T // 512         # 4 token chunks
F32 = mybir.dt.float32
BF16 = mybir.dt.bfloat16

_NC_CACHE = None


def _build():
    nc = bacc.Bacc("TRN2", target_bir_lowering=False, debug=False)
    xt = nc.dram_tensor("xt", [C, T], BF16, kind="ExternalInput").ap()
    wqkvt = nc.dram_tensor("wqkvt", [C, 3 * F], BF16, kind="ExternalInput").ap()
    woutt = nc.dram_tensor("woutt", [F, C], BF16, kind="ExternalInput").ap()
    out = nc.dram_tensor("out", [C, T], F32, kind="ExternalOutput").ap()

    with ExitStack() as ctx:
        tc = ctx.enter_context(tile.TileContext(nc))
        qk = ctx.enter_context(tc.tile_pool(name="qk", bufs=1))
        vp = ctx.enter_context(tc.tile_pool(name="vp", bufs=1))
        wqp = ctx.enter_context(tc.tile_pool(name="wqp", bufs=1))
        wop = ctx.enter_context(tc.tile_pool(name="wop", bufs=1))
        xcp = ctx.enter_context(tc.tile_pool(name="xcp", bufs=2))
        pbp = ctx.enter_context(tc.tile_pool(name="pbp", bufs=8))
        rp = ctx.enter_context(tc.tile_pool(name="rp", bufs=4))
        rbp = ctx.enter_context(tc.tile_pool(name="rbp", bufs=4))
        ytp = ctx.enter_context(tc.tile_pool(name="ytp", bufs=2))
        oop = ctx.enter_context(tc.tile_pool(name="oop", bufs=3))
        # PSUM: scores 4 banks (2 kt x 2 heads, single-buffered: the WAR
        # against the previous super-step's exp provides the pipelining
        # at 2-kt granularity) + AV pair accumulators 2 + proj/outproj 2
        sps = ctx.enter_context(tc.tile_pool(name="sps", bufs=1, space="PSUM"))
        yps = ctx.enter_context(tc.tile_pool(name="yps", bufs=1, space="PSUM"))
        gps = ctx.enter_context(tc.tile_pool(name="gps", bufs=2, space="PSUM"))

        # SBUF-resident tensors for the whole kernel:
        #   qT[m][128f, T], kT[m][128f, T] bf16 feature-major
        #   vt[tm][128tk, 583] bf16 token-major, 8 head-groups of 65 cols
        #   (64 v features + ones col), tail-padded so every 128-col FWL
        #   weight window stays in bounds; pad/ones cols only feed psum
        #   partitions >= 65 which are never read.
        qts = [qk.tile([128, T], BF16, tag=f"q{m}", name=f"q{m}") for m in range(4)]
        kts = [qk.tile([128, T], BF16, tag=f"k{m}", name=f"k{m}") for m in range(4)]
        vts = [vp.tile([128, 583], BF16, tag=f"v{tm}", name=f"v{tm}")
               for tm in range(T // 128)]
        wq = [wqp.tile([128, 3 * F], BF16, tag=f"w{ki}", name=f"w{ki}")
              for ki in range(KI)]
        wo = [wop.tile([128, C], BF16, tag=f"wo{ki}", name=f"wo{ki}")
              for ki in range(F // 128)]

        # ---- DMA prologue: q/k weight halves + chunk-0 x first (in use
        # order so the first matmuls start after the first pair lands),
        # v weight halves next; wo is deferred to the qc loop ----
        xc0 = []
        for ki in range(KI):
            nc.sync.dma_start(out=wq[ki][:, 0:2 * F],
                              in_=wqkvt[ki * 128:(ki + 1) * 128, 0:2 * F])
            t = xcp.tile([128, 512], BF16, tag=f"xc{ki}", name=f"xc{ki}")
            nc.sync.dma_start(out=t[:], in_=xt[ki * 128:(ki + 1) * 128, 0:512])
            xc0.append(t)
        for ki in range(KI):
            nc.sync.dma_start(out=wq[ki][:, 2 * F:3 * F],
                              in_=wqkvt[ki * 128:(ki + 1) * 128, 2 * F:3 * F])
        for tm in range(T // 128):
            nc.gpsimd.memset(vts[tm][:], 1.0)
        # initialize the scores psum banks once: diagonal super-steps exp
        # columns the odd kt's matmul didn't write (results never read by AV)
        ps0 = sps.tile([128, 2, 2, 512], F32, tag="ps", name="ps_init")
        nc.vector.memset(ps0[:], 0.0)

        # ---- projection emitters ----
        def proj_qk(xc, n, ms):
            # q/k feature-major: psum tile per m, ki-major accumulation
            pts = [gps.tile([128, 512], F32, tag="gp", name=f"gp{m}") for m in ms]
            for ki in range(KI):
                for i, m in enumerate(ms):
                    nc.tensor.matmul(pts[i][:], wq[ki][:, m * 128:(m + 1) * 128],
                                     xc[ki][:], start=(ki == 0), stop=(ki == KI - 1))
            for i, m in enumerate(ms):
                dst = (qts[m] if m < 4 else kts[m - 4])[:, n * 512:(n + 1) * 512]
                nc.vector.tensor_copy(dst, pts[i][:])

        def proj_v(xc, n, tmis):
            # v token-major
            pts = [gps.tile([128, 512], F32, tag="gp", name=f"gpv{t}") for t in tmis]
            for ki in range(KI):
                for i, tmi in enumerate(tmis):
                    nc.tensor.matmul(pts[i][:], xc[ki][:, tmi * 128:(tmi + 1) * 128],
                                     wq[ki][:, 2 * F:3 * F],
                                     start=(ki == 0), stop=(ki == KI - 1))
            for i, tmi in enumerate(tmis):
                tm = n * 4 + tmi
                vdst = vts[tm][:, 0:520].rearrange("p (h c) -> p h c", c=65)
                nc.vector.tensor_copy(
                    vdst[:, :, 0:64], pts[i][:].rearrange("p (h c) -> p h c", c=64))

        # ---- filler machinery: PE work to interleave into attention ----
        fillers = deque()          # (cost_ns, fn, tag)

        def pump(ns):
            while ns > 0 and fillers:
                c, f, _ = fillers.popleft()
                f()
                ns -= c

        def flush_tag(tag):
            while any(e[2] == tag for e in fillers):
                _, f, _ = fillers.popleft()
                f()

        def flush():
            while fillers:
                fillers.popleft()[1]()

        def mk_proj_qk(xc, n, m):
            return (KI * 223 + 50, lambda: proj_qk(xc, n, [m]), f"c{n}h{m % 4}")

        def mk_proj_v(xc, n, tmi):
            return (KI * 223 + 50, lambda: proj_v(xc, n, [tmi]), f"c{n}v")

        def mk_outproj(yy, qc, m):
            def emit():
                po = gps.tile([128, 512], F32, tag="gp", name="gpo")
                for ki in range(F // 128):
                    nc.tensor.matmul(po[:], wo[ki][:, m * 128:(m + 1) * 128],
                                     yy[ki][:], start=(ki == 0),
                                     stop=(ki == F // 128 - 1))
                oo = oop.tile([128, 512], F32, tag="oo", name="oo")
                nc.vector.tensor_copy(oo[:], po[:])
                nc.sync.dma_start(
                    out=out[m * 128:(m + 1) * 128, qc * 512:(qc + 1) * 512],
                    in_=oo[:])
            return (4 * 223 + 50, emit, f"op{qc}")

        # chunk-0 projection: hp0's q/k and all of v inline (ki-major 2-psum
        # groups so compute starts with the first DMA pair); remaining q/k
        # groups become fillers gated per head-pair
        proj_qk(xc0, 0, [0, 4])
        proj_v(xc0, 0, [0, 1])
        proj_v(xc0, 0, [2, 3])
        held = {}
        held[0] = [(2 * KI * 223 + 50,
                    (lambda h: lambda: proj_qk(xc0, 0, [h, h + 4]))(hp),
                    f"c0h{hp}") for hp in (1, 2, 3)]

        pending_norm = [None]

        def emit_pending_norm():
            if pending_norm[0] is not None:
                pending_norm[0]()
                pending_norm[0] = None

        # ---- fused attention + interleaved proj/out-proj ----
        for qc in range(NT):
            # deferred q/k projection units of chunk qc (emitted inside this
            # slot, gated per head-pair) -- keeps PE filler available in the
            # ACT-bound late slots
            fillers.extend(held.pop(qc, []))
            if qc + 1 < NT:
                xcn = []
                for ki in range(KI):
                    t = xcp.tile([128, 512], BF16, tag=f"xc{ki}", name=f"xc{ki}")
                    nc.sync.dma_start(
                        out=t[:],
                        in_=xt[ki * 128:(ki + 1) * 128,
                               (qc + 1) * 512:(qc + 2) * 512])
                    xcn.append(t)
                # v + head-pair-0 groups are due by the qc boundary; the
                # other head-pairs' groups are held for slot qc+1 itself
                for tmi in range(4):
                    fillers.append(mk_proj_v(xcn, qc + 1, tmi))
                fillers.append(mk_proj_qk(xcn, qc + 1, 0))
                fillers.append(mk_proj_qk(xcn, qc + 1, 4))
                held[qc + 1] = [mk_proj_qk(xcn, qc + 1, m)
                                for m in (1, 5, 2, 6, 3, 7)]
            if qc == 0:
                # wo needed first by outproj(0), pumped during qc1
                for ki in range(F // 128):
                    nc.sync.dma_start(out=wo[ki][:],
                                      in_=woutt[ki * 128:(ki + 1) * 128, :])

            n_kt = qc * 4 + 4
            yy = [ytp.tile([128, 512], BF16, tag=f"y{i}", name=f"y{i}")
                  for i in range(4)]
            for hp in range(HPC // 2):       # head pairs (2*hp, 2*hp+1)
                if hp > 0:
                    flush_tag(f"c{qc}h{hp}")
                qpair = qts[hp][:, qc * 512:(qc + 1) * 512]
                pyA = yps.tile([128, 512], F32, tag="pyA", name="pyA")
                pyB = yps.tile([128, 512], F32, tag="pyB", name="pyB")

                def emit_av(item, hp=hp, n_kt=n_kt, pyA=pyA, pyB=pyB):
                    kt, lo, pb, i = item
                    a0 = 2 * hp * 65
                    nc.tensor.matmul(pyA[:, lo:512], vts[kt][:, a0:a0 + 128],
                                     pb[:, i, 0, lo:512],
                                     start=(kt == 0), stop=(kt == n_kt - 1))
                    nc.tensor.matmul(pyB[:, lo:512], vts[kt][:, a0 + 65:a0 + 193],
                                     pb[:, i, 1, lo:512],
                                     start=(kt == 0), stop=(kt == n_kt - 1))

                pending_av = deque()
                for j in range(n_kt // 2):
                    # super-step: scores for kt=2j,2j+1 into one 4-bank psum
                    # tile, ONE exp instruction covering both (halves the ACT
                    # per-instruction overhead). For the diagonal super-step
                    # the odd kt's columns [lo_e, lo_o) hold stale psum; exp
                    # of them is finite garbage that the AV matmuls never read.
                    ps = sps.tile([128, 2, 2, 512], F32, tag="ps", name="ps")
                    los = []
                    for i, kt in enumerate((2 * j, 2 * j + 1)):
                        ksl = kts[hp][:, kt * 128:(kt + 1) * 128]
                        lo = max((kt - qc * 4) * 128, 0)
                        los.append(lo)
                        nc.tensor.matmul(ps[:, i, 0, lo:512], ksl[0:64, :],
                                         qpair[0:64, lo:512], start=True,
                                         stop=True, tile_position=(0, 0))
                        nc.tensor.matmul(ps[:, i, 1, lo:512], ksl[64:128, :],
                                         qpair[64:128, lo:512], start=True,
                                         stop=True, tile_position=(64, 0))
                    lo_e = los[0]
                    pb = pbp.tile([128, 2, 2, 512], BF16, tag="pb", name="pb")
                    nc.scalar.activation(pb[:, :, :, lo_e:512],
                                         ps[:, :, :, lo_e:512],
                                         mybir.ActivationFunctionType.Exp,
                                         scale=0.125)
                    for i, kt in enumerate((2 * j, 2 * j + 1)):
                        lo = los[i]
                        if kt >= qc * 4:
                            # zero the strictly-lower triangle (query < key)
                            # of the diagonal 128x128 block, post-exp
                            reg = pb[:, i, :, lo:lo + 128]
                            nc.gpsimd.affine_select(
                                out=reg, in_=reg,
                                compare_op=mybir.AluOpType.is_ge, fill=0.0,
                                base=0, pattern=[[0, 2], [1, 128]],
                                channel_multiplier=-1)
                        pending_av.append((kt, lo, pb, i))
                    if j == 0:
                        emit_pending_norm()
                    while len(pending_av) > 2:
                        emit_av(pending_av.popleft())
                    n2 = (512 - los[0]) + (512 - los[1])
                    act_ns = (2 * n2 + 352) / 1.2 + (300 if los[1] else 0)
                    pe_ns = 3 * n2 / 2.4 + 120
                    pump(act_ns - pe_ns)
                emit_pending_norm()
                while pending_av:
                    emit_av(pending_av.popleft())

                def norm(hp=hp, qc=qc, pyA=pyA, pyB=pyB, yy=yy):
                    # denominators sit in psum partition 64 (ones-column of V);
                    # custom-DVE recip can't read PSUM on HW: bounce via SBUF
                    rA = rp.tile([1, 512], F32, tag="rA", name="rA")
                    rB = rp.tile([1, 512], F32, tag="rB", name="rB")
                    nc.vector.tensor_copy(rA[:], pyA[64:65, :])
                    nc.vector.tensor_copy(rB[:], pyB[64:65, :])
                    nc.vector.reciprocal_approx_fast(out=rA[:], in_=rA[:])
                    nc.vector.reciprocal_approx_fast(out=rB[:], in_=rB[:])
                    rbA = rbp.tile([64, 512], F32, tag="rbA", name="rbA")
                    rbB = rbp.tile([64, 512], F32, tag="rbB", name="rbB")
                    nc.gpsimd.partition_broadcast(rbA[:], rA[:])
                    nc.gpsimd.partition_broadcast(rbB[:], rB[:])
                    nc.vector.tensor_mul(yy[hp][0:64, :], pyA[0:64, :], rbA[:])
                    nc.vector.tensor_mul(yy[hp][64:128, :], pyB[0:64, :], rbB[:])

                pending_norm[0] = norm
                if hp + 1 < HPC // 2:
                    # next pair's deferred proj, with lead time for its evac
                    flush_tag(f"c{qc}h{hp + 1}")
                pump(1200)
            # chunk qc+1's v and head-pair-0 groups must be complete before
            # slot qc+1 starts; outproj(qc-1) must drain before yy bufs
            # recycle; the last pair's norm must be emitted before outproj(qc)
            # fillers (Tile deps are emission-order based)
            emit_pending_norm()
            if qc + 1 < NT:
                flush_tag(f"c{qc + 1}v")
                flush_tag(f"c{qc + 1}h0")
            if qc > 0:
                flush_tag(f"op{qc - 1}")
            for m in range(8):
                fillers.append(mk_outproj(yy, qc, m))
        flush()
    nc.finalize()
    return nc


def _get_nc():
    global _NC_CACHE
    if _NC_CACHE is None:
        _NC_CACHE = _build()
    return _NC_CACHE


def kernel(x, w_qkv, w_out):
    x = np.ascontiguousarray(np.asarray(x), dtype=np.float32)
    w_qkv = np.asarray(w_qkv, dtype=np.float32)
    w_out = np.asarray(w_out, dtype=np.float32)
    nc = _get_nc()

    in_maps = []
    for c in range(NCORES):
        b, j = divmod(c, 2)
        rows = np.r_[j * F:(j + 1) * F,
                     C + j * F:C + (j + 1) * F,
                     2 * C + j * F:2 * C + (j + 1) * F]
        in_maps.append({
            "xt": np.ascontiguousarray(x[b].T).astype(ml_dtypes.bfloat16),
            "wqkvt": np.ascontiguousarray(w_qkv[rows, :].T).astype(ml_dtypes.bfloat16),
            "woutt": np.ascontiguousarray(
                w_out[:, j * F:(j + 1) * F].T).astype(ml_dtypes.bfloat16),
        })

    res = run_bass_kernel_spmd(nc, in_maps, core_ids=list(range(NCORES)))
    y = np.empty((B, T, C), np.float32)
    for b in range(B):
        y[b] = (res.results[2 * b]["out"] + res.results[2 * b + 1]["out"]).T
    return y


# revision 22
# speedup vs baseline: 1.1015x; 1.0718x over previous
"""Causal self-attention (B=4, T=2048, C=1024, H=16) on 8 TRN2 NeuronCores.

Sharding: tensor-parallel pairs. Core c handles batch b = c//2 and head-half
j = c%2 (8 of the 16 heads). Each core computes the QKV projection for its
heads, causal attention, and the out-projection contracted over its half of
the features, producing a partial output. The pair-sum (the "all-reduce after
out_proj" of the tensor-parallel scheme) happens at unshard time on the host.

Single fused pipeline, all matmuls bf16 (fp32 PSUM accumulate). The QKV
projection and out-projection matmuls are emitted as *filler* between
attention kt-steps so the PE stream stays dense (HAM stays at 8/8) while
ScalarE's exp stream (~155us, the attention-phase bound) overlaps fully.

Causal handling: exact 128-column trim per diagonal tile (lo = d*128), the
1/sqrt(D) scale rides the ACT instruction's free scale field, and the
diagonal triangle of the probs is zeroed on GpSimd after exp (scaled scores
are ~N(0,1) so exp of unmasked garbage can't overflow) -- keeping VectorE
off the scores->exp critical path. Softmax denominators come free from a
ones-column appended to V so the AV matmul accumulates sum(exp) in PSUM.
The per-pair normalization chain (copy denom -> reciprocal -> broadcast ->
scale) is emitted lazily inside the NEXT pair's kt loop so its ~3us latency
hides under scores/filler work instead of stalling the PSUM accumulators.
"""
import ml_dtypes
import numpy as np
from collections import deque
from contextlib import ExitStack

import concourse.bass as bass
from concourse import bacc
import concourse.mybir as mybir
import concourse.tile as tile
from concourse.bass_utils import run_bass_kernel_spmd

B, T, C, H, D = 4, 2048, 1024, 16, 64
NCORES = 8
HPC = H // 2          # heads per core
F = HPC * D           # 512 features per core (per q/k/v)
KI = C // 128         # 8 contraction tiles over C
NT = T // 512         # 4 token chunks
F32 = mybir.dt.float32
BF16 = mybir.dt.bfloat16

_NC_CACHE = None


def _build():
    nc = bacc.Bacc("TRN2", target_bir_lowering=False, debug=False)
    xt = nc.dram_tensor("xt", [C, T], BF16, kind="ExternalInput").ap()
    wqkvt = nc.dram_tensor("wqkvt", [C, 3 * F], BF16, kind="ExternalInput").ap()
    woutt = nc.dram_tensor("woutt", [F, C], BF16, kind="ExternalInput").ap()
    out = nc.dram_tensor("out", [C, T], F32, kind="ExternalOutput").ap()

    with ExitStack() as ctx:
        tc = ctx.enter_context(tile.TileContext(nc))
        qk = ctx.enter_context(tc.tile_pool(name="qk", bufs=1))
        vp = ctx.enter_context(tc.tile_pool(name="vp", bufs=1))
        wqp = ctx.enter_context(tc.tile_pool(name="wqp", bufs=1))
        wop = ctx.enter_context(tc.tile_pool(name="wop", bufs=1))
        xcp = ctx.enter_context(tc.tile_pool(name="xcp", bufs=2))
        pbp = ctx.enter_context(tc.tile_pool(name="pbp", bufs=8))
        rp = ctx.enter_context(tc.tile_pool(name="rp", bufs=4))
        rbp = ctx.enter_context(tc.tile_pool(name="rbp", bufs=4))
        ytp = ctx.enter_context(tc.tile_pool(name="ytp", bufs=3))
        oop = ctx.enter_context(tc.tile_pool(name="oop", bufs=3))
        # PSUM: scores 2x2 banks (per-kt tiles double-buffered: exp(k) overlaps
        # scores(k+1), unlike 2-kt-batched exp which serializes into a
        # ping-pong) + AV pair accumulators 2 + shared proj/outproj 2
        sps = ctx.enter_context(tc.tile_pool(name="sps", bufs=2, space="PSUM"))
        yps = ctx.enter_context(tc.tile_pool(name="yps", bufs=1, space="PSUM"))
        gps = ctx.enter_context(tc.tile_pool(name="gps", bufs=2, space="PSUM"))

        # SBUF-resident tensors for the whole kernel:
        #   qT[m][128f, T], kT[m][128f, T] bf16 feature-major
        #   vt[tm][128tk, 583] bf16 token-major, 8 head-groups of 65 cols
        #   (64 v features + ones col), tail-padded so every 128-col FWL
        #   weight window stays in bounds; pad/ones cols only feed psum
        #   partitions >= 65 which are never read.
        qts = [qk.tile([128, T], BF16, tag=f"q{m}", name=f"q{m}") for m in range(4)]
        kts = [qk.tile([128, T], BF16, tag=f"k{m}", name=f"k{m}") for m in range(4)]
        vts = [vp.tile([128, 583], BF16, tag=f"v{tm}", name=f"v{tm}")
               for tm in range(T // 128)]
        wq = [wqp.tile([128, 3 * F], BF16, tag=f"w{ki}", name=f"w{ki}")
              for ki in range(KI)]
        wo = [wop.tile([128, C], BF16, tag=f"wo{ki}", name=f"wo{ki}")
              for ki in range(F // 128)]

        # ---- DMA prologue: q/k weight halves + chunk-0 x first (in use
        # order so the first matmuls start after the first pair lands),
        # v weight halves next; wo is deferred to the qc loop ----
        xc0 = []
        for ki in range(KI):
            nc.sync.dma_start(out=wq[ki][:, 0:2 * F],
                              in_=wqkvt[ki * 128:(ki + 1) * 128, 0:2 * F])
            t = xcp.tile([128, 512], BF16, tag=f"xc{ki}", name=f"xc{ki}")
            nc.sync.dma_start(out=t[:], in_=xt[ki * 128:(ki + 1) * 128, 0:512])
            xc0.append(t)
        for ki in range(KI):
            nc.sync.dma_start(out=wq[ki][:, 2 * F:3 * F],
                              in_=wqkvt[ki * 128:(ki + 1) * 128, 2 * F:3 * F])
        for tm in range(T // 128):
            nc.gpsimd.memset(vts[tm][:], 1.0)

        # ---- projection emitters ----
        def proj_qk(xc, n, ms):
            # q/k feature-major: psum tile per m, ki-major accumulation
            pts = [gps.tile([128, 512], F32, tag="gp", name=f"gp{m}") for m in ms]
            for ki in range(KI):
                for i, m in enumerate(ms):
                    nc.tensor.matmul(pts[i][:], wq[ki][:, m * 128:(m + 1) * 128],
                                     xc[ki][:], start=(ki == 0), stop=(ki == KI - 1))
            for i, m in enumerate(ms):
                dst = (qts[m] if m < 4 else kts[m - 4])[:, n * 512:(n + 1) * 512]
                nc.vector.tensor_copy(dst, pts[i][:])

        def proj_v(xc, n, tmis):
            # v token-major
            pts = [gps.tile([128, 512], F32, tag="gp", name=f"gpv{t}") for t in tmis]
            for ki in range(KI):
                for i, tmi in enumerate(tmis):
                    nc.tensor.matmul(pts[i][:], xc[ki][:, tmi * 128:(tmi + 1) * 128],
                                     wq[ki][:, 2 * F:3 * F],
                                     start=(ki == 0), stop=(ki == KI - 1))
            for i, tmi in enumerate(tmis):
                tm = n * 4 + tmi
                vdst = vts[tm][:, 0:520].rearrange("p (h c) -> p h c", c=65)
                nc.vector.tensor_copy(
                    vdst[:, :, 0:64], pts[i][:].rearrange("p (h c) -> p h c", c=64))

        # ---- filler machinery: PE work to interleave into attention ----
        fillers = deque()          # (cost_ns, fn, tag)

        def pump(ns):
            while ns > 0 and fillers:
                c, f, _ = fillers.popleft()
                f()
                ns -= c

        def flush_tag(tag):
            while any(e[2] == tag for e in fillers):
                _, f, _ = fillers.popleft()
                f()

        def flush():
            while fillers:
                fillers.popleft()[1]()

        def mk_proj_qk(xc, n, m):
            return (KI * 223 + 50, lambda: proj_qk(xc, n, [m]), f"c{n}h{m % 4}")

        def mk_proj_v(xc, n, tmi):
            return (KI * 223 + 50, lambda: proj_v(xc, n, [tmi]), f"c{n}v")

        def mk_outproj(yy, qc, m):
            def emit():
                po = gps.tile([128, 512], F32, tag="gp", name="gpo")
                for ki in range(F // 128):
                    nc.tensor.matmul(po[:], wo[ki][:, m * 128:(m + 1) * 128],
                                     yy[ki][:], start=(ki == 0),
                                     stop=(ki == F // 128 - 1))
                oo = oop.tile([128, 512], F32, tag="oo", name="oo")
                nc.vector.tensor_copy(oo[:], po[:])
                nc.sync.dma_start(
                    out=out[m * 128:(m + 1) * 128, qc * 512:(qc + 1) * 512],
                    in_=oo[:])
            return (4 * 223 + 50, emit, f"op{qc}")

        # chunk-0 projection: hp0's q/k and all of v inline (ki-major 2-psum
        # groups so compute starts with the first DMA pair); remaining q/k
        # groups become fillers gated per head-pair
        proj_qk(xc0, 0, [0, 4])
        proj_v(xc0, 0, [0, 1])
        proj_v(xc0, 0, [2, 3])
        held = {}
        held[0] = [(2 * KI * 223 + 50,
                    (lambda h: lambda: proj_qk(xc0, 0, [h, h + 4]))(hp),
                    f"c0h{hp}") for hp in (1, 2, 3)]

        pending_norm = [None]

        def emit_pending_norm():
            if pending_norm[0] is not None:
                pending_norm[0]()
                pending_norm[0] = None

        # ---- fused attention + interleaved proj/out-proj ----
        for qc in range(NT):
            # deferred q/k projection units of chunk qc (emitted inside this
            # slot, gated per head-pair) -- keeps PE filler available in the
            # ACT-bound late slots
            fillers.extend(held.pop(qc, []))
            if qc + 1 < NT:
                xcn = []
                for ki in range(KI):
                    t = xcp.tile([128, 512], BF16, tag=f"xc{ki}", name=f"xc{ki}")
                    nc.sync.dma_start(
                        out=t[:],
                        in_=xt[ki * 128:(ki + 1) * 128,
                               (qc + 1) * 512:(qc + 2) * 512])
                    xcn.append(t)
                # v + head-pair-0 groups are due by the qc boundary; the
                # other head-pairs' groups are held for slot qc+1 itself
                for tmi in range(4):
                    fillers.append(mk_proj_v(xcn, qc + 1, tmi))
                fillers.append(mk_proj_qk(xcn, qc + 1, 0))
                fillers.append(mk_proj_qk(xcn, qc + 1, 4))
                held.setdefault(qc + 1, []).extend(
                    mk_proj_qk(xcn, qc + 1, m) for m in (1, 5, 2, 6, 3, 7))
            if qc == 0:
                # wo needed first by outproj(0), pumped during qc1
                for ki in range(F // 128):
                    nc.sync.dma_start(out=wo[ki][:],
                                      in_=woutt[ki * 128:(ki + 1) * 128, :])

            n_kt = qc * 4 + 4
            yy = [ytp.tile([128, 512], BF16, tag=f"y{i}", name=f"y{i}")
                  for i in range(4)]
            for hp in range(HPC // 2):       # head pairs (2*hp, 2*hp+1)
                if hp > 0:
                    flush_tag(f"c{qc}h{hp}")
                qpair = qts[hp][:, qc * 512:(qc + 1) * 512]
                pyA = yps.tile([128, 512], F32, tag="pyA", name="pyA")
                pyB = yps.tile([128, 512], F32, tag="pyB", name="pyB")

                def emit_av(item, hp=hp, n_kt=n_kt, pyA=pyA, pyB=pyB):
                    kt, lo, pb = item
                    a0 = 2 * hp * 65
                    nc.tensor.matmul(pyA[:, lo:512], vts[kt][:, a0:a0 + 128],
                                     pb[:, 0, lo:512],
                                     start=(kt == 0), stop=(kt == n_kt - 1))
                    nc.tensor.matmul(pyB[:, lo:512], vts[kt][:, a0 + 65:a0 + 193],
                                     pb[:, 1, lo:512],
                                     start=(kt == 0), stop=(kt == n_kt - 1))

                pending_av = deque()
                for kt in range(n_kt):
                    ksl = kts[hp][:, kt * 128:(kt + 1) * 128]
                    # exact causal trim: cols < lo are fully masked
                    lo = max((kt - qc * 4) * 128, 0)
                    ps = sps.tile([128, 2, 512], F32, tag="ps", name="ps")
                    nc.tensor.matmul(ps[:, 0, lo:512], ksl[0:64, :],
                                     qpair[0:64, lo:512],
                                     start=True, stop=True, tile_position=(0, 0))
                    nc.tensor.matmul(ps[:, 1, lo:512], ksl[64:128, :],
                                     qpair[64:128, lo:512],
                                     start=True, stop=True, tile_position=(64, 0))
                    pb = pbp.tile([128, 2, 512], BF16, tag="pb", name="pb")
                    nc.scalar.activation(pb[:, :, lo:512], ps[:, :, lo:512],
                                         mybir.ActivationFunctionType.Exp,
                                         scale=0.125)
                    diag = kt >= qc * 4
                    if diag:
                        # zero the strictly-lower triangle (query < key) of the
                        # diagonal 128x128 block, post-exp, off the DVE path
                        reg = pb[:, :, lo:lo + 128]
                        nc.gpsimd.affine_select(
                            out=reg, in_=reg,
                            compare_op=mybir.AluOpType.is_ge, fill=0.0,
                            base=0, pattern=[[0, 2], [1, 128]],
                            channel_multiplier=-1)
                    pending_av.append((kt, lo, pb))
                    if kt == 1:
                        emit_pending_norm()
                    act_ns = (2 * (512 - lo) + 352) / 1.2 + (150 if diag else 0)
                    pe_ns = 3 * (512 - lo) / 2.4 + 60
                    # fillers between scores and the lagged AV so the PE queue
                    # head never sits directly on a fresh exp's result
                    pump(300)
                    # AV lag 4 at pair start clears the previous pair's norm
                    # chain (pyA/pyB WAR); steady-state lag 2
                    lag = 4 if kt < 6 else 2
                    while len(pending_av) > lag:
                        emit_av(pending_av.popleft())
                    pump(act_ns - pe_ns - 150)
                emit_pending_norm()
                while pending_av:
                    emit_av(pending_av.popleft())
                    pump(250)

                def norm(hp=hp, qc=qc, pyA=pyA, pyB=pyB, yy=yy):
                    # denominators sit in psum partition 64 (ones-column of V);
                    # custom-DVE recip can't read PSUM on HW: bounce via SBUF
                    rA = rp.tile([1, 512], F32, tag="rA", name="rA")
                    rB = rp.tile([1, 512], F32, tag="rB", name="rB")
                    nc.vector.tensor_copy(rA[:], pyA[64:65, :])
                    nc.vector.tensor_copy(rB[:], pyB[64:65, :])
                    nc.vector.reciprocal_approx_fast(out=rA[:], in_=rA[:])
                    nc.vector.reciprocal_approx_fast(out=rB[:], in_=rB[:])
                    rbA = rbp.tile([64, 512], F32, tag="rbA", name="rbA")
                    rbB = rbp.tile([64, 512], F32, tag="rbB", name="rbB")
                    nc.gpsimd.partition_broadcast(rbA[:], rA[:])
                    nc.gpsimd.partition_broadcast(rbB[:], rB[:])
                    nc.vector.tensor_mul(yy[hp][0:64, :], pyA[0:64, :], rbA[:])
                    nc.vector.tensor_mul(yy[hp][64:128, :], pyB[0:64, :], rbB[:])

                pending_norm[0] = norm
                if hp + 1 < HPC // 2:
                    # next pair's deferred proj, with lead time for its evac
                    flush_tag(f"c{qc}h{hp + 1}")
                pump(1200)
            # chunk qc+1's v and head-pair-0 groups must be complete before
            # slot qc+1 starts; outproj(qc) drifts late (held for the
            # ACT-bound slots; yy bufs=3 gives a 2-slot deadline); the last
            # pair's norm must be emitted before outproj(qc) units
            emit_pending_norm()
            if qc + 1 < NT:
                flush_tag(f"c{qc + 1}v")
                flush_tag(f"c{qc + 1}h0")
            if qc >= 2:
                flush_tag(f"op{qc - 2}")
            ops = [mk_outproj(yy, qc, m) for m in range(8)]
            if qc == 0:
                held.setdefault(2, []).extend(ops)
            elif qc < NT - 1:
                held.setdefault(3, []).extend(ops)
            else:
                fillers.extend(ops)
        flush()
    nc.finalize()
    return nc


def _get_nc():
    global _NC_CACHE
    if _NC_CACHE is None:
        _NC_CACHE = _build()
    return _NC_CACHE


def kernel(x, w_qkv, w_out):
    x = np.ascontiguousarray(np.asarray(x), dtype=np.float32)
    w_qkv = np.asarray(w_qkv, dtype=np.float32)
    w_out = np.asarray(w_out, dtype=np.float32)
    nc = _get_nc()

    in_maps = []
    for c in range(NCORES):
        b, j = divmod(c, 2)
        rows = np.r_[j * F:(j + 1) * F,
                     C + j * F:C + (j + 1) * F,
                     2 * C + j * F:2 * C + (j + 1) * F]
        in_maps.append({
            "xt": np.ascontiguousarray(x[b].T).astype(ml_dtypes.bfloat16),
            "wqkvt": np.ascontiguousarray(w_qkv[rows, :].T).astype(ml_dtypes.bfloat16),
            "woutt": np.ascontiguousarray(
                w_out[:, j * F:(j + 1) * F].T).astype(ml_dtypes.bfloat16),
        })

    res = run_bass_kernel_spmd(nc, in_maps, core_ids=list(range(NCORES)))
    y = np.empty((B, T, C), np.float32)
    for b in range(B):
        y[b] = (res.results[2 * b]["out"] + res.results[2 * b + 1]["out"]).T
    return y
